# revision 8
# baseline (speedup 1.0000x reference)
"""Trainium2 Bass kernel: masked (sparse-adjacency) attention.

Computes, for full inputs:
    adj    = adjs[idx]                      # [Na, N] bool
    scores = (anchor @ wt) @ x.T            # [Na, N]
    atten  = softmax(where(adj, scores, -inf) / T, axis=1)
    out    = weight[idx] * (atten @ x)      # [Na, d_out]

Sharding: anchors (rows of the score matrix) are split across the 8
NeuronCores, 1280 rows per core (Na=10000 padded to 10240). x / wt are
replicated; the adjacency is shipped pre-transposed per shard.

Per-core device pipeline over j-tiles of 128 x-rows (N=10000 padded to
10112 = 79*128), software-pipelined so the in-order PE queue never
blocks the ACT engine (the bottleneck):

  iter j:  S(j)   = X_j^T.T @ Q^T    (PE -> PSUM s-half j%2)
           P(j)   = exp(S(j)/T)      (ACT, one 1280-wide instr, bf16)
           PM(j)  = P(j) * adjT(j)   (DVE bf16 2x)
           AV(j-1): O^T += Xaug.T @ PM(j-1)  (PE, accumulating PSUM)

PSUM layout (8 banks exactly): one [128, 2560] f32 score tile (5 banks)
manually split into two 1280-wide halves for double buffering - the
matmul chunk widths adapt to each half's bank alignment (512|512|256 for
even j at byte 0, 256|512|512 for odd j at byte 5120) so every matmul
output stays inside one 2KB bank - plus the [65, 1280] f32 output
accumulator (3 banks). The single wide exp per j-tile amortizes the
~352-cycle per-instruction ACT overhead.

Masking happens after exp as a multiply by {0,1}: exp(s/T) is bounded
(|s| < ~2 for this data), and multiplying by the adjacency bit zeroes
masked entries exactly, which is mathematically identical to softmax
over the masked entries. The ones column appended to x yields the
softmax denominators in the same accumulated GEMM.
"""

import numpy as np
import ml_dtypes

import concourse.bacc as bacc
import concourse.bass as bass
import concourse.mybir as mybir
import concourse.tile as tile
from concourse.bass_utils import run_bass_kernel_spmd

F32 = mybir.dt.float32
F32R = mybir.dt.float32r  # fp32 fast-path: 1 PE cycle/col at N>=256
BF16 = mybir.dt.bfloat16

N_CORES = 8
N = 10000          # x rows (softmax width)
NA = 10000         # anchors
D_IN = 256
D_OUT = 64
TEMP = 0.07

NJ_TILES = 79                 # ceil(10000 / 128)
NJ = NJ_TILES * 128           # 10112, padded x-rows
A_CORE = 1280                 # anchors per core (10240 padded / 8)
M_AUG = D_OUT + 1             # 65: d_out columns + ones column

# matmul chunk widths per s-half: each chunk's PSUM write must stay
# inside one 2KB bank. half 0 starts at byte 0 (bank-aligned), half 1
# at byte 5120 (mid-bank), so its first chunk is the 256 remainder.
S_CHUNKS = (
    ((0, 512), (512, 512), (1024, 256)),
    ((0, 256), (256, 512), (768, 512)),
)
AV_CHUNKS = ((0, 512), (512, 512), (1024, 256))


def _build_bass():
    # Bacc (not plain Bass): its compile() runs the wait-splitting passes
    # (move_matmul_waits_to_ldweights / generate_event_semaphores) that
    # keep every instruction within the TRN2 1-sync-wait ISA limit.
    nc = bacc.Bacc(
        "TRN2",
        target_bir_lowering=False,
        debug=False,
        num_devices=N_CORES,
    )
    xT = nc.dram_tensor("xT", [D_OUT, NJ], F32R, kind="ExternalInput").ap()
    xaug = nc.dram_tensor(
        "xaug", [128, NJ_TILES * M_AUG], BF16, kind="ExternalInput"
    ).ap()
    anchT = nc.dram_tensor("anchT", [D_IN, A_CORE], F32R, kind="ExternalInput").ap()
    adjT = nc.dram_tensor("adjT", [NJ, A_CORE], BF16, kind="ExternalInput").ap()
    wt = nc.dram_tensor("wt", [D_IN, D_OUT], F32R, kind="ExternalInput").ap()
    wscale = nc.dram_tensor("wscale", [128, 1], F32, kind="ExternalInput").ap()
    ident = nc.dram_tensor("ident", [128, 128], F32, kind="ExternalInput").ap()
    out = nc.dram_tensor("out", [A_CORE, D_OUT], F32, kind="ExternalOutput").ap()

    EXP = mybir.ActivationFunctionType.Exp

    with tile.TileContext(nc) as tc:
        with tc.tile_pool(name="const", bufs=1) as const:
            # small Q^T-feeding tensors first so the prologue matmuls can
            # start while the big xT/xaug loads still stream
            ident_sb = const.tile([128, 128], F32)
            nc.sync.dma_start(ident_sb[:], ident[:])
            wscale_sb = const.tile([128, 1], F32)
            nc.sync.dma_start(wscale_sb[:], wscale[:])
            xT_sb = const.tile([D_OUT, NJ], F32R)
            xaug_sb = const.tile([128, NJ_TILES * M_AUG], BF16)
            qt_sb = const.tile([D_OUT, A_CORE], F32R)
            ot_sb = const.tile([M_AUG, A_CORE], F32)

            # ---- Q^T = wt.T @ anchor.T  -> [64, 1280] ----
            with (
                tc.tile_pool(name="pre", bufs=1) as pre,
                tc.tile_pool(name="prepsum", bufs=1, space="PSUM") as prepsum,
            ):
                an0 = pre.tile([128, A_CORE], F32R)
                nc.sync.dma_start(an0[:], anchT[0:128, :])
                an1 = pre.tile([128, A_CORE], F32R)
                nc.sync.dma_start(an1[:], anchT[128:256, :])
                wt0 = pre.tile([128, D_OUT], F32R)
                nc.sync.dma_start(wt0[:], wt[0:128, :])
                wt1 = pre.tile([128, D_OUT], F32R)
                nc.sync.dma_start(wt1[:], wt[128:256, :])
                # big streaming loads issued after the Q^T operands
                nc.sync.dma_start(xT_sb[:], xT[:])
                nc.sync.dma_start(xaug_sb[:], xaug[:])
                qt_ps = prepsum.tile([D_OUT, A_CORE], F32)
                for n0, nw in AV_CHUNKS:
                    nc.tensor.matmul(
                        qt_ps[:, n0 : n0 + nw],
                        wt0[:],
                        an0[:, n0 : n0 + nw],
                        start=True,
                        stop=False,
                    )
                    nc.tensor.matmul(
                        qt_ps[:, n0 : n0 + nw],
                        wt1[:],
                        an1[:, n0 : n0 + nw],
                        start=False,
                        stop=True,
                    )
                nc.vector.tensor_copy(qt_sb[:], qt_ps[:])

            # ---- main software-pipelined loop over j tiles ----
            with (
                tc.tile_pool(name="adjp", bufs=4) as adjp,
                tc.tile_pool(name="pp", bufs=3) as pp,
                tc.tile_pool(name="pmp", bufs=3) as pmp,
                tc.tile_pool(name="spsum", bufs=1, space="PSUM") as spsum,
                tc.tile_pool(name="opsum", bufs=1, space="PSUM") as opsum,
            ):
                # [128, 2560] = 5 PSUM banks; halves at bytes 0 / 5120
                s_full = spsum.tile([128, 2 * A_CORE], F32)
                o_ps = opsum.tile([M_AUG, A_CORE], F32)

                adj_tiles = {}

                def fetch_adj(j):
                    t = adjp.tile([128, A_CORE], BF16, name="adj_t")
                    nc.sync.dma_start(t[:], adjT[j * 128 : (j + 1) * 128, :])
                    adj_tiles[j] = t

                fetch_adj(0)
                fetch_adj(1)

                pm_tiles = {}
                for j in range(NJ_TILES + 1):
                    if j < NJ_TILES:
                        if j + 2 < NJ_TILES:
                            fetch_adj(j + 2)
                        half = j % 2
                        s_sl = s_full[:, half * A_CORE : (half + 1) * A_CORE]
                        xt_w = xT_sb[:, j * 128 : (j + 1) * 128]
                        for n0, nw in S_CHUNKS[half]:
                            nc.tensor.matmul(
                                s_sl[:, n0 : n0 + nw],
                                xt_w,
                                qt_sb[:, n0 : n0 + nw],
                                start=True,
                                stop=True,
                            )
                        p_t = pp.tile([128, A_CORE], BF16, name="p_t")
                        nc.scalar.activation(p_t[:], s_sl[:], EXP, scale=1.0 / TEMP)
                        pm_t = pmp.tile([128, A_CORE], BF16, name="pm_t")
                        nc.vector.tensor_mul(pm_t[:], p_t[:], adj_tiles.pop(j)[:])
                        pm_tiles[j] = pm_t
                        # dummy weight loads: pure PE activity with no
                        # architectural effect (each matmul self-loads its
                        # weights). They fill the PE's dependency-wait gaps
                        # so the HAM clock gate sees continuous activity and
                        # keeps the PE at 2.4 GHz instead of 1.2 GHz.
                        for _ in range(2):
                            nc.tensor.ldweights(xaug_sb[:, 0:128])
                    if j >= 1:
                        # AV for the previous tile: emitted after S(j) so
                        # the in-order PE queue stays ahead of ACT
                        ja = j - 1
                        pm_prev = pm_tiles.pop(ja)
                        xa_w = xaug_sb[:, ja * M_AUG : (ja + 1) * M_AUG]
                        for n0, nw in AV_CHUNKS:
                            nc.tensor.matmul(
                                o_ps[:, n0 : n0 + nw],
                                xa_w,
                                pm_prev[:, n0 : n0 + nw],
                                start=(ja == 0),
                                stop=(ja == NJ_TILES - 1),
                            )
                        for _ in range(2):
                            nc.tensor.ldweights(xaug_sb[:, 0:128])
                nc.vector.tensor_copy(ot_sb[:], o_ps[:])

            # ---- tail: transpose back, normalize, scale, store ----
            with (
                tc.tile_pool(name="tpsum", bufs=2, space="PSUM") as tpsum,
                tc.tile_pool(name="tail", bufs=2) as tail,
            ):
                for k in range(A_CORE // 128):
                    t_ps = tpsum.tile([128, M_AUG], F32)
                    nc.tensor.transpose(
                        t_ps[:],
                        ot_sb[0:M_AUG, k * 128 : (k + 1) * 128],
                        ident_sb[0:M_AUG, 0:M_AUG],
                    )
                    rec = tail.tile([128, 1], F32)
                    nc.vector.reciprocal(rec[:], t_ps[:, D_OUT : D_OUT + 1])
                    rec2 = tail.tile([128, 1], F32)
                    nc.vector.tensor_mul(rec2[:], rec[:], wscale_sb[:])
                    o_t = tail.tile([128, D_OUT], F32)
                    nc.vector.tensor_scalar_mul(o_t[:], t_ps[:, 0:D_OUT], rec2[:])
                    nc.sync.dma_start(out[k * 128 : (k + 1) * 128, :], o_t[:])

    nc.compile()
    return nc


def _prep_inputs(x, weight, adjs, idx, anchor, wt):
    i = int(np.asarray(idx))
    x = np.asarray(x, dtype=np.float32)
    anchor = np.asarray(anchor, dtype=np.float32)
    wt = np.asarray(wt, dtype=np.float32)
    adj = np.asarray(adjs)[i]  # [Na, N] bool
    w = float(np.asarray(weight)[i])

    NAP = N_CORES * A_CORE  # 10240

    xT = np.zeros((D_OUT, NJ), dtype=np.float32)
    xT[:, :N] = x.T

    xaug = np.zeros((NJ, M_AUG), dtype=ml_dtypes.bfloat16)
    xaug[:N, :D_OUT] = x
    xaug[:N, D_OUT] = 1.0
    xaug_strip = np.ascontiguousarray(
        xaug.reshape(NJ_TILES, 128, M_AUG).transpose(1, 0, 2).reshape(128, -1)
    )

    anchorT = np.zeros((D_IN, NAP), dtype=np.float32)
    anchorT[:, :NA] = anchor.T

    # adjacency, transposed to [N, Na], as bf16 {0.0, 1.0}
    adj_u16 = np.zeros((NJ, NAP), dtype=np.uint16)
    adj_u16[:N, :NA] = adj.T
    adj_u16 *= 0x3F80  # bf16 bit pattern of 1.0
    # padded anchor columns: one fake edge to x-row 0 so denominators are
    # finite (those rows are discarded on the host)
    adj_u16[0, NA:] = 0x3F80
    adj_bf = adj_u16.view(ml_dtypes.bfloat16)

    ident = np.eye(128, dtype=np.float32)
    wscale = np.full((128, 1), w, dtype=np.float32)

    in_maps = []
    for c in range(N_CORES):
        sl = slice(c * A_CORE, (c + 1) * A_CORE)
        in_maps.append(
            {
                "xT": xT,
                "xaug": xaug_strip,
                "anchT": np.ascontiguousarray(anchorT[:, sl]),
                "adjT": np.ascontiguousarray(adj_bf[:, sl]),
                "wt": wt,
                "wscale": wscale,
                "ident": ident,
            }
        )
    return in_maps


def run(x, weight, adjs, idx, anchor, wt, trace=False, **spmd_kwargs):
    in_maps = _prep_inputs(x, weight, adjs, idx, anchor, wt)
    nc = _build_bass()
    res = run_bass_kernel_spmd(
        nc, in_maps, core_ids=list(range(N_CORES)), trace=trace, **spmd_kwargs
    )
    out = np.concatenate([r["out"] for r in res.results], axis=0)[:NA]
    return np.ascontiguousarray(out.astype(np.float32)), res


def kernel(x, weight, adjs, idx, anchor, wt):
    out, _ = run(x, weight, adjs, idx, anchor, wt)
    return out


# revision 9
# speedup vs baseline: 1.0417x; 1.0417x over previous
"""Trainium2 Bass kernel: masked (sparse-adjacency) attention.

Computes, for full inputs:
    adj    = adjs[idx]                      # [Na, N] bool
    scores = (anchor @ wt) @ x.T            # [Na, N]
    atten  = softmax(where(adj, scores, -inf) / T, axis=1)
    out    = weight[idx] * (atten @ x)      # [Na, d_out]

Sharding: anchors (rows of the score matrix) are split across the 8
NeuronCores, 1280 rows per core (Na=10000 padded to 10240). x / wt are
replicated; the adjacency is shipped pre-transposed per shard.

Per-core device pipeline over j-tiles of 128 x-rows (N=10000 padded to
10112 = 79*128), software-pipelined so the in-order PE queue never
blocks the ACT engine (the bottleneck):

  iter j:  S(j)   = X_j^T.T @ Q^T    (PE -> PSUM s-half j%2)
           P(j)   = exp(S(j)/T)      (ACT, one 1280-wide instr, bf16)
           PM(j)  = P(j) * adjT(j)   (DVE bf16 2x)
           AV(j-1): O^T += Xaug.T @ PM(j-1)  (PE, accumulating PSUM)

PSUM layout (8 banks exactly): one [128, 2560] f32 score tile (5 banks)
manually split into two 1280-wide halves for double buffering - the
matmul chunk widths adapt to each half's bank alignment (512|512|256 for
even j at byte 0, 256|512|512 for odd j at byte 5120) so every matmul
output stays inside one 2KB bank - plus the [65, 1280] f32 output
accumulator (3 banks). The single wide exp per j-tile amortizes the
~352-cycle per-instruction ACT overhead.

Masking happens after exp as a multiply by {0,1}: exp(s/T) is bounded
(|s| < ~2 for this data), and multiplying by the adjacency bit zeroes
masked entries exactly, which is mathematically identical to softmax
over the masked entries. The ones column appended to x yields the
softmax denominators in the same accumulated GEMM.
"""

import numpy as np
import ml_dtypes

import concourse.bacc as bacc
import concourse.bass as bass
import concourse.mybir as mybir
import concourse.tile as tile
from concourse.bass_utils import run_bass_kernel_spmd

F32 = mybir.dt.float32
F32R = mybir.dt.float32r  # fp32 fast-path: 1 PE cycle/col at N>=256
BF16 = mybir.dt.bfloat16

N_CORES = 8
N = 10000          # x rows (softmax width)
NA = 10000         # anchors
D_IN = 256
D_OUT = 64
TEMP = 0.07

NJ_TILES = 79                 # ceil(10000 / 128)
NJ = NJ_TILES * 128           # 10112, padded x-rows
A_CORE = 1280                 # anchors per core (10240 padded / 8)
M_AUG = D_OUT + 1             # 65: d_out columns + ones column

# matmul chunk widths per s-half: each chunk's PSUM write must stay
# inside one 2KB bank. half 0 starts at byte 0 (bank-aligned), half 1
# at byte 5120 (mid-bank), so its first chunk is the 256 remainder.
S_CHUNKS = (
    ((0, 512), (512, 512), (1024, 256)),
    ((0, 256), (256, 512), (768, 512)),
)
AV_CHUNKS = ((0, 512), (512, 512), (1024, 256))


def _build_bass():
    # Bacc (not plain Bass): its compile() runs the wait-splitting passes
    # (move_matmul_waits_to_ldweights / generate_event_semaphores) that
    # keep every instruction within the TRN2 1-sync-wait ISA limit.
    nc = bacc.Bacc(
        "TRN2",
        target_bir_lowering=False,
        debug=False,
        num_devices=N_CORES,
    )
    xT = nc.dram_tensor("xT", [D_OUT, NJ], F32R, kind="ExternalInput").ap()
    xaug = nc.dram_tensor(
        "xaug", [128, NJ_TILES * M_AUG], BF16, kind="ExternalInput"
    ).ap()
    anchT = nc.dram_tensor("anchT", [D_IN, A_CORE], F32R, kind="ExternalInput").ap()
    adjT = nc.dram_tensor("adjT", [NJ, A_CORE], BF16, kind="ExternalInput").ap()
    wt = nc.dram_tensor("wt", [D_IN, D_OUT], F32R, kind="ExternalInput").ap()
    wscale = nc.dram_tensor("wscale", [128, 1], F32, kind="ExternalInput").ap()
    ident = nc.dram_tensor("ident", [128, 128], F32, kind="ExternalInput").ap()
    out = nc.dram_tensor("out", [A_CORE, D_OUT], F32, kind="ExternalOutput").ap()

    EXP = mybir.ActivationFunctionType.Exp

    with tile.TileContext(nc) as tc:
        with tc.tile_pool(name="const", bufs=1) as const:
            # small Q^T-feeding tensors first so the prologue matmuls can
            # start while the big xT/xaug loads still stream
            ident_sb = const.tile([128, 128], F32)
            nc.sync.dma_start(ident_sb[:], ident[:])
            wscale_sb = const.tile([128, 1], F32)
            nc.sync.dma_start(wscale_sb[:], wscale[:])
            xT_sb = const.tile([D_OUT, NJ], F32R)
            xaug_sb = const.tile([128, NJ_TILES * M_AUG], BF16)
            qt_sb = const.tile([D_OUT, A_CORE], F32R)
            ot_sb = const.tile([M_AUG, A_CORE], F32)

            # ---- Q^T = wt.T @ anchor.T  -> [64, 1280] ----
            with (
                tc.tile_pool(name="pre", bufs=1) as pre,
                tc.tile_pool(name="prepsum", bufs=1, space="PSUM") as prepsum,
            ):
                an0 = pre.tile([128, A_CORE], F32R)
                nc.sync.dma_start(an0[:], anchT[0:128, :])
                an1 = pre.tile([128, A_CORE], F32R)
                nc.sync.dma_start(an1[:], anchT[128:256, :])
                wt0 = pre.tile([128, D_OUT], F32R)
                nc.sync.dma_start(wt0[:], wt[0:128, :])
                wt1 = pre.tile([128, D_OUT], F32R)
                nc.sync.dma_start(wt1[:], wt[128:256, :])
                # big streaming loads issued after the Q^T operands
                nc.sync.dma_start(xT_sb[:], xT[:])
                nc.sync.dma_start(xaug_sb[:], xaug[:])
                warm_ps = prepsum.tile([128, 128], F32)
                for _ in range(12):
                    nc.tensor.matmul(
                        warm_ps[:], ident_sb[:], ident_sb[:], start=True, stop=True
                    )
                qt_ps = prepsum.tile([D_OUT, A_CORE], F32)
                for n0, nw in AV_CHUNKS:
                    nc.tensor.matmul(
                        qt_ps[:, n0 : n0 + nw],
                        wt0[:],
                        an0[:, n0 : n0 + nw],
                        start=True,
                        stop=False,
                    )
                    nc.tensor.matmul(
                        qt_ps[:, n0 : n0 + nw],
                        wt1[:],
                        an1[:, n0 : n0 + nw],
                        start=False,
                        stop=True,
                    )
                nc.vector.tensor_copy(qt_sb[:], qt_ps[:])

            # ---- main software-pipelined loop over j tiles ----
            with (
                tc.tile_pool(name="adjp", bufs=4) as adjp,
                tc.tile_pool(name="pp", bufs=3) as pp,
                tc.tile_pool(name="pmp", bufs=3) as pmp,
                tc.tile_pool(name="spsum", bufs=1, space="PSUM") as spsum,
                tc.tile_pool(name="opsum", bufs=1, space="PSUM") as opsum,
            ):
                # [128, 2560] = 5 PSUM banks; halves at bytes 0 / 5120
                s_full = spsum.tile([128, 2 * A_CORE], F32)
                o_ps = opsum.tile([M_AUG, A_CORE], F32)

                adj_tiles = {}

                def fetch_adj(j):
                    t = adjp.tile([128, A_CORE], BF16, name="adj_t")
                    nc.sync.dma_start(t[:], adjT[j * 128 : (j + 1) * 128, :])
                    adj_tiles[j] = t

                fetch_adj(0)
                fetch_adj(1)

                pm_tiles = {}
                for j in range(NJ_TILES + 1):
                    if j < NJ_TILES:
                        if j + 2 < NJ_TILES:
                            fetch_adj(j + 2)
                        half = j % 2
                        s_sl = s_full[:, half * A_CORE : (half + 1) * A_CORE]
                        xt_w = xT_sb[:, j * 128 : (j + 1) * 128]
                        for n0, nw in S_CHUNKS[half]:
                            nc.tensor.matmul(
                                s_sl[:, n0 : n0 + nw],
                                xt_w,
                                qt_sb[:, n0 : n0 + nw],
                                start=True,
                                stop=True,
                            )
                        p_t = pp.tile([128, A_CORE], BF16, name="p_t")
                        nc.scalar.activation(p_t[:], s_sl[:], EXP, scale=1.0 / TEMP)
                        pm_t = pmp.tile([128, A_CORE], BF16, name="pm_t")
                        nc.vector.tensor_mul(pm_t[:], p_t[:], adj_tiles.pop(j)[:])
                        pm_tiles[j] = pm_t
                    if j >= 1:
                        # AV for the previous tile: emitted after S(j) so
                        # the in-order PE queue stays ahead of ACT
                        ja = j - 1
                        pm_prev = pm_tiles.pop(ja)
                        xa_w = xaug_sb[:, ja * M_AUG : (ja + 1) * M_AUG]
                        for n0, nw in AV_CHUNKS:
                            nc.tensor.matmul(
                                o_ps[:, n0 : n0 + nw],
                                xa_w,
                                pm_prev[:, n0 : n0 + nw],
                                start=(ja == 0),
                                stop=(ja == NJ_TILES - 1),
                            )
                nc.vector.tensor_copy(ot_sb[:], o_ps[:])

            # ---- tail: transpose back, normalize, scale, store ----
            with (
                tc.tile_pool(name="tpsum", bufs=2, space="PSUM") as tpsum,
                tc.tile_pool(name="tail", bufs=2) as tail,
            ):
                for k in range(A_CORE // 128):
                    t_ps = tpsum.tile([128, M_AUG], F32)
                    nc.tensor.transpose(
                        t_ps[:],
                        ot_sb[0:M_AUG, k * 128 : (k + 1) * 128],
                        ident_sb[0:M_AUG, 0:M_AUG],
                    )
                    rec = tail.tile([128, 1], F32)
                    nc.vector.reciprocal(rec[:], t_ps[:, D_OUT : D_OUT + 1])
                    rec2 = tail.tile([128, 1], F32)
                    nc.vector.tensor_mul(rec2[:], rec[:], wscale_sb[:])
                    o_t = tail.tile([128, D_OUT], F32)
                    nc.vector.tensor_scalar_mul(o_t[:], t_ps[:, 0:D_OUT], rec2[:])
                    nc.sync.dma_start(out[k * 128 : (k + 1) * 128, :], o_t[:])

    nc.compile()
    return nc


def _prep_inputs(x, weight, adjs, idx, anchor, wt):
    i = int(np.asarray(idx))
    x = np.asarray(x, dtype=np.float32)
    anchor = np.asarray(anchor, dtype=np.float32)
    wt = np.asarray(wt, dtype=np.float32)
    adj = np.asarray(adjs)[i]  # [Na, N] bool
    w = float(np.asarray(weight)[i])

    NAP = N_CORES * A_CORE  # 10240

    xT = np.zeros((D_OUT, NJ), dtype=np.float32)
    xT[:, :N] = x.T

    xaug = np.zeros((NJ, M_AUG), dtype=ml_dtypes.bfloat16)
    xaug[:N, :D_OUT] = x
    xaug[:N, D_OUT] = 1.0
    xaug_strip = np.ascontiguousarray(
        xaug.reshape(NJ_TILES, 128, M_AUG).transpose(1, 0, 2).reshape(128, -1)
    )

    anchorT = np.zeros((D_IN, NAP), dtype=np.float32)
    anchorT[:, :NA] = anchor.T

    # adjacency, transposed to [N, Na], as bf16 {0.0, 1.0}
    adj_u16 = np.zeros((NJ, NAP), dtype=np.uint16)
    adj_u16[:N, :NA] = adj.T
    adj_u16 *= 0x3F80  # bf16 bit pattern of 1.0
    # padded anchor columns: one fake edge to x-row 0 so denominators are
    # finite (those rows are discarded on the host)
    adj_u16[0, NA:] = 0x3F80
    adj_bf = adj_u16.view(ml_dtypes.bfloat16)

    ident = np.eye(128, dtype=np.float32)
    wscale = np.full((128, 1), w, dtype=np.float32)

    in_maps = []
    for c in range(N_CORES):
        sl = slice(c * A_CORE, (c + 1) * A_CORE)
        in_maps.append(
            {
                "xT": xT,
                "xaug": xaug_strip,
                "anchT": np.ascontiguousarray(anchorT[:, sl]),
                "adjT": np.ascontiguousarray(adj_bf[:, sl]),
                "wt": wt,
                "wscale": wscale,
                "ident": ident,
            }
        )
    return in_maps


def run(x, weight, adjs, idx, anchor, wt, trace=False, **spmd_kwargs):
    in_maps = _prep_inputs(x, weight, adjs, idx, anchor, wt)
    nc = _build_bass()
    res = run_bass_kernel_spmd(
        nc, in_maps, core_ids=list(range(N_CORES)), trace=trace, **spmd_kwargs
    )
    out = np.concatenate([r["out"] for r in res.results], axis=0)[:NA]
    return np.ascontiguousarray(out.astype(np.float32)), res


def kernel(x, weight, adjs, idx, anchor, wt):
    out, _ = run(x, weight, adjs, idx, anchor, wt)
    return out


# revision 10
# speedup vs baseline: 1.0643x; 1.0218x over previous
"""Trainium2 Bass kernel: masked (sparse-adjacency) attention.

Computes, for full inputs:
    adj    = adjs[idx]                      # [Na, N] bool
    scores = (anchor @ wt) @ x.T            # [Na, N]
    atten  = softmax(where(adj, scores, -inf) / T, axis=1)
    out    = weight[idx] * (atten @ x)      # [Na, d_out]

Sharding: anchors (rows of the score matrix) are split across the 8
NeuronCores, 1280 rows per core (Na=10000 padded to 10240). x / wt are
replicated; the adjacency is shipped pre-transposed per shard.

Per-core device pipeline over j-tiles of 128 x-rows (N=10000 padded to
10112 = 79*128), software-pipelined so the in-order PE queue never
blocks the ACT engine (the bottleneck):

  iter j:  S(j)   = X_j^T.T @ Q^T    (PE -> PSUM s-half j%2)
           P(j)   = exp(S(j)/T)      (ACT, one 1280-wide instr, bf16)
           PM(j)  = P(j) * adjT(j)   (DVE bf16 2x)
           AV(j-1): O^T += Xaug.T @ PM(j-1)  (PE, accumulating PSUM)

PSUM layout (8 banks exactly): one [128, 2560] f32 score tile (5 banks)
manually split into two 1280-wide halves for double buffering - the
matmul chunk widths adapt to each half's bank alignment (512|512|256 for
even j at byte 0, 256|512|512 for odd j at byte 5120) so every matmul
output stays inside one 2KB bank - plus the [65, 1280] f32 output
accumulator (3 banks). The single wide exp per j-tile amortizes the
~352-cycle per-instruction ACT overhead.

Masking happens after exp as a multiply by {0,1}: exp(s/T) is bounded
(|s| < ~2 for this data), and multiplying by the adjacency bit zeroes
masked entries exactly, which is mathematically identical to softmax
over the masked entries. The ones column appended to x yields the
softmax denominators in the same accumulated GEMM.
"""

import numpy as np
import ml_dtypes

import concourse.bacc as bacc
import concourse.bass as bass
import concourse.mybir as mybir
import concourse.tile as tile
from concourse.bass_utils import run_bass_kernel_spmd

F32 = mybir.dt.float32
F32R = mybir.dt.float32r  # fp32 fast-path: 1 PE cycle/col at N>=256
BF16 = mybir.dt.bfloat16

N_CORES = 8
N = 10000          # x rows (softmax width)
NA = 10000         # anchors
D_IN = 256
D_OUT = 64
TEMP = 0.07

NJ_TILES = 79                 # ceil(10000 / 128)
NJ = NJ_TILES * 128           # 10112, padded x-rows
A_CORE = 1280                 # anchors per core (10240 padded / 8)
M_AUG = D_OUT + 1             # 65: d_out columns + ones column

# matmul chunk widths per s-half: each chunk's PSUM write must stay
# inside one 2KB bank. half 0 starts at byte 0 (bank-aligned), half 1
# at byte 5120 (mid-bank), so its first chunk is the 256 remainder.
S_CHUNKS = (
    ((0, 512), (512, 512), (1024, 256)),
    ((0, 256), (256, 512), (768, 512)),
)
AV_CHUNKS = ((0, 512), (512, 512), (1024, 256))


def _build_bass():
    # Bacc (not plain Bass): its compile() runs the wait-splitting passes
    # (move_matmul_waits_to_ldweights / generate_event_semaphores) that
    # keep every instruction within the TRN2 1-sync-wait ISA limit.
    nc = bacc.Bacc(
        "TRN2",
        target_bir_lowering=False,
        debug=False,
        num_devices=N_CORES,
    )
    xT = nc.dram_tensor("xT", [D_OUT, NJ], F32R, kind="ExternalInput").ap()
    xaug = nc.dram_tensor(
        "xaug", [128, NJ_TILES * M_AUG], BF16, kind="ExternalInput"
    ).ap()
    anchT = nc.dram_tensor("anchT", [D_IN, A_CORE], F32R, kind="ExternalInput").ap()
    adjT = nc.dram_tensor("adjT", [NJ, A_CORE], BF16, kind="ExternalInput").ap()
    wt = nc.dram_tensor("wt", [D_IN, D_OUT], F32R, kind="ExternalInput").ap()
    wscale = nc.dram_tensor("wscale", [128, 1], F32, kind="ExternalInput").ap()
    ident = nc.dram_tensor("ident", [128, 128], F32, kind="ExternalInput").ap()
    out = nc.dram_tensor("out", [A_CORE, D_OUT], F32, kind="ExternalOutput").ap()

    EXP = mybir.ActivationFunctionType.Exp

    with tile.TileContext(nc) as tc:
        with tc.tile_pool(name="const", bufs=1) as const:
            # small Q^T-feeding tensors first so the prologue matmuls can
            # start while the big xT/xaug loads still stream
            ident_sb = const.tile([128, 128], F32)
            nc.sync.dma_start(ident_sb[:], ident[:])
            wscale_sb = const.tile([128, 1], F32)
            nc.sync.dma_start(wscale_sb[:], wscale[:])
            xT_sb = const.tile([D_OUT, NJ], F32R)
            xaug_sb = const.tile([128, NJ_TILES * M_AUG], BF16)
            qt_sb = const.tile([D_OUT, A_CORE], F32R)
            ot_sb = const.tile([M_AUG, A_CORE], F32)

            # ---- Q^T = wt.T @ anchor.T  -> [64, 1280] ----
            with (
                tc.tile_pool(name="pre", bufs=1) as pre,
                tc.tile_pool(name="prepsum", bufs=1, space="PSUM") as prepsum,
            ):
                an0 = pre.tile([128, A_CORE], F32R)
                nc.sync.dma_start(an0[:], anchT[0:128, :])
                an1 = pre.tile([128, A_CORE], F32R)
                nc.sync.dma_start(an1[:], anchT[128:256, :])
                wt0 = pre.tile([128, D_OUT], F32R)
                nc.sync.dma_start(wt0[:], wt[0:128, :])
                wt1 = pre.tile([128, D_OUT], F32R)
                nc.sync.dma_start(wt1[:], wt[128:256, :])
                # big streaming loads, split into strips: the main loop
                # only waits on the strip it is about to read
                XS = 16 * 128
                nc.sync.dma_start(xT_sb[:, 0:XS], xT[:, 0:XS])
                nc.sync.dma_start(xaug_sb[:, 0 : 16 * M_AUG], xaug[:, 0 : 16 * M_AUG])
                for c0 in range(XS, NJ, XS):
                    c1 = min(c0 + XS, NJ)
                    nc.sync.dma_start(xT_sb[:, c0:c1], xT[:, c0:c1])
                for c0 in range(16 * M_AUG, NJ_TILES * M_AUG, 16 * M_AUG):
                    c1 = min(c0 + 16 * M_AUG, NJ_TILES * M_AUG)
                    nc.sync.dma_start(xaug_sb[:, c0:c1], xaug[:, c0:c1])
                warm_ps = prepsum.tile([128, 128], F32)
                for _ in range(32):
                    nc.tensor.matmul(
                        warm_ps[:], ident_sb[:], ident_sb[:], start=True, stop=True
                    )
                qt_ps = prepsum.tile([D_OUT, A_CORE], F32)
                for n0, nw in AV_CHUNKS:
                    nc.tensor.matmul(
                        qt_ps[:, n0 : n0 + nw],
                        wt0[:],
                        an0[:, n0 : n0 + nw],
                        start=True,
                        stop=False,
                    )
                    nc.tensor.matmul(
                        qt_ps[:, n0 : n0 + nw],
                        wt1[:],
                        an1[:, n0 : n0 + nw],
                        start=False,
                        stop=True,
                    )
                nc.vector.tensor_copy(qt_sb[:], qt_ps[:])

            # ---- main software-pipelined loop over j tiles ----
            with (
                tc.tile_pool(name="adjp", bufs=4) as adjp,
                tc.tile_pool(name="pp", bufs=3) as pp,
                tc.tile_pool(name="pmp", bufs=3) as pmp,
                tc.tile_pool(name="spsum", bufs=1, space="PSUM") as spsum,
                tc.tile_pool(name="opsum", bufs=1, space="PSUM") as opsum,
            ):
                # [128, 2560] = 5 PSUM banks; halves at bytes 0 / 5120
                s_full = spsum.tile([128, 2 * A_CORE], F32)
                o_ps = opsum.tile([M_AUG, A_CORE], F32)

                adj_tiles = {}

                def fetch_adj(j):
                    t = adjp.tile([128, A_CORE], BF16, name="adj_t")
                    nc.sync.dma_start(t[:], adjT[j * 128 : (j + 1) * 128, :])
                    adj_tiles[j] = t

                fetch_adj(0)
                fetch_adj(1)

                pm_tiles = {}
                for j in range(NJ_TILES + 1):
                    if j < NJ_TILES:
                        if j + 2 < NJ_TILES:
                            fetch_adj(j + 2)
                        half = j % 2
                        s_sl = s_full[:, half * A_CORE : (half + 1) * A_CORE]
                        xt_w = xT_sb[:, j * 128 : (j + 1) * 128]
                        for n0, nw in S_CHUNKS[half]:
                            nc.tensor.matmul(
                                s_sl[:, n0 : n0 + nw],
                                xt_w,
                                qt_sb[:, n0 : n0 + nw],
                                start=True,
                                stop=True,
                            )
                        p_t = pp.tile([128, A_CORE], BF16, name="p_t")
                        nc.scalar.activation(p_t[:], s_sl[:], EXP, scale=1.0 / TEMP)
                        pm_t = pmp.tile([128, A_CORE], BF16, name="pm_t")
                        nc.vector.tensor_mul(pm_t[:], p_t[:], adj_tiles.pop(j)[:])
                        pm_tiles[j] = pm_t
                    if j >= 1:
                        # AV for the previous tile: emitted after S(j) so
                        # the in-order PE queue stays ahead of ACT
                        ja = j - 1
                        pm_prev = pm_tiles.pop(ja)
                        xa_w = xaug_sb[:, ja * M_AUG : (ja + 1) * M_AUG]
                        for n0, nw in AV_CHUNKS:
                            nc.tensor.matmul(
                                o_ps[:, n0 : n0 + nw],
                                xa_w,
                                pm_prev[:, n0 : n0 + nw],
                                start=(ja == 0),
                                stop=(ja == NJ_TILES - 1),
                            )
                nc.vector.tensor_copy(ot_sb[:], o_ps[:])

            # ---- tail: transpose back, normalize, scale, store ----
            with (
                tc.tile_pool(name="tpsum", bufs=2, space="PSUM") as tpsum,
                tc.tile_pool(name="tail", bufs=2) as tail,
            ):
                for k in range(A_CORE // 128):
                    t_ps = tpsum.tile([128, M_AUG], F32)
                    nc.tensor.transpose(
                        t_ps[:],
                        ot_sb[0:M_AUG, k * 128 : (k + 1) * 128],
                        ident_sb[0:M_AUG, 0:M_AUG],
                    )
                    rec = tail.tile([128, 1], F32)
                    nc.vector.reciprocal(rec[:], t_ps[:, D_OUT : D_OUT + 1])
                    rec2 = tail.tile([128, 1], F32)
                    nc.vector.tensor_mul(rec2[:], rec[:], wscale_sb[:])
                    o_t = tail.tile([128, D_OUT], F32)
                    nc.vector.tensor_scalar_mul(o_t[:], t_ps[:, 0:D_OUT], rec2[:])
                    nc.sync.dma_start(out[k * 128 : (k + 1) * 128, :], o_t[:])

    nc.compile()
    return nc


def _prep_inputs(x, weight, adjs, idx, anchor, wt):
    i = int(np.asarray(idx))
    x = np.asarray(x, dtype=np.float32)
    anchor = np.asarray(anchor, dtype=np.float32)
    wt = np.asarray(wt, dtype=np.float32)
    adj = np.asarray(adjs)[i]  # [Na, N] bool
    w = float(np.asarray(weight)[i])

    NAP = N_CORES * A_CORE  # 10240

    xT = np.zeros((D_OUT, NJ), dtype=np.float32)
    xT[:, :N] = x.T

    xaug = np.zeros((NJ, M_AUG), dtype=ml_dtypes.bfloat16)
    xaug[:N, :D_OUT] = x
    xaug[:N, D_OUT] = 1.0
    xaug_strip = np.ascontiguousarray(
        xaug.reshape(NJ_TILES, 128, M_AUG).transpose(1, 0, 2).reshape(128, -1)
    )

    anchorT = np.zeros((D_IN, NAP), dtype=np.float32)
    anchorT[:, :NA] = anchor.T

    # adjacency, transposed to [N, Na], as bf16 {0.0, 1.0}
    adj_u16 = np.zeros((NJ, NAP), dtype=np.uint16)
    adj_u16[:N, :NA] = adj.T
    adj_u16 *= 0x3F80  # bf16 bit pattern of 1.0
    # padded anchor columns: one fake edge to x-row 0 so denominators are
    # finite (those rows are discarded on the host)
    adj_u16[0, NA:] = 0x3F80
    adj_bf = adj_u16.view(ml_dtypes.bfloat16)

    ident = np.eye(128, dtype=np.float32)
    wscale = np.full((128, 1), w, dtype=np.float32)

    in_maps = []
    for c in range(N_CORES):
        sl = slice(c * A_CORE, (c + 1) * A_CORE)
        in_maps.append(
            {
                "xT": xT,
                "xaug": xaug_strip,
                "anchT": np.ascontiguousarray(anchorT[:, sl]),
                "adjT": np.ascontiguousarray(adj_bf[:, sl]),
                "wt": wt,
                "wscale": wscale,
                "ident": ident,
            }
        )
    return in_maps


def run(x, weight, adjs, idx, anchor, wt, trace=False, **spmd_kwargs):
    in_maps = _prep_inputs(x, weight, adjs, idx, anchor, wt)
    nc = _build_bass()
    res = run_bass_kernel_spmd(
        nc, in_maps, core_ids=list(range(N_CORES)), trace=trace, **spmd_kwargs
    )
    out = np.concatenate([r["out"] for r in res.results], axis=0)[:NA]
    return np.ascontiguousarray(out.astype(np.float32)), res


def kernel(x, weight, adjs, idx, anchor, wt):
    out, _ = run(x, weight, adjs, idx, anchor, wt)
    return out
